# revision 8
# baseline (speedup 1.0000x reference)
"""Multi-head attention (B=2, S=2048, D=1024, H=16) on 8 Trainium2 NeuronCores.

Sharding: data-parallel over batch (groups of 4 cores) x sequence-parallel
attention inside each group.
  core c -> batch g = c // 4, sequence block r = c % 4 (rows r*512..r*512+512).

Per core (own rows = own 512 queries = own 512 keys):
  phase A (projections; every matmul is K=128 N=512 off the same xq tiles):
    kT_part [1024, 512] = wq-style wk_full.T @ xq   (K, all heads, own keys)
    V_own   [512, 1024] = xq.T @ wv_full            (V, all heads, own rows)
    qT_own  [1024, 512] = wq_full.T @ xq            (Q, all heads, own queries)
  Two early AllGathers per 4-core group, triggered as soon as their producer
  wave finishes: K first (it feeds the long exp chain), then V.
  Key tiles are RELABELED per core (block b = group-rank (r+b)%4) so tiles
  0..3 are always the core's own keys: attention on them needs no gather and
  the SPMD program stays compile-time static; softmax is order-invariant.
  phase B (attention, all 16 heads x own 512 queries x 2048 keys):
    per head pair, per local key-tile pair: scoresT = kT-tile.T @ q with the
    q halves zero-padded per head (K=128, shared stationary operand), one
    N=1024 exp per head on ScalarE, PV accumulate with an appended ones
    column collecting the softmax denominator (row 64 of the PSUM acc).
  phase C: local output projection out = aoT.T @ w_proj + b_proj (no
  collective on the tail).
Host-side work is only slicing/transposition of inputs and concatenation of
outputs.
"""

import os
import sys

import numpy as np

try:
    import ml_dtypes

    BF16_NP = ml_dtypes.bfloat16
except ImportError:  # pragma: no cover
    BF16_NP = None

for _p in ("/opt/trn_rl_repo",):
    if os.path.isdir(_p) and _p not in sys.path:
        sys.path.append(_p)

import concourse.bass as bass  # noqa: E402
import concourse.mybir as mybir  # noqa: E402
import concourse.tile as tile  # noqa: E402
from concourse import bacc  # noqa: E402
from concourse.bass_utils import run_bass_kernel_spmd  # noqa: E402

B, S, D = 2, 2048, 1024
H, HD = 16, 64
N_CORES = 8
GROUP = 4
S_OWN = S // GROUP  # 512 own rows (queries and keys)
N_KT = D // 128  # 8 contraction tiles
N_SK = S // 128  # 16 key tiles
PAIRS = H // 2  # 8 head pairs

FP32 = mybir.dt.float32
FP32R = mybir.dt.float32r
BF16 = mybir.dt.bfloat16

_compiled = None
_ONES = np.ones((128, 128), dtype=np.float32)
_ONES16 = None

# packed f32 per-partition constants: [bqa 0:8 | bqb 8:16 | bk 16:24 |
#  maska 24 | maskb 25]
FC_BQA, FC_BQB, FC_BK, FC_MA, FC_MB, FC_W = 0, 8, 16, 24, 25, 26


def _build():
    nc = bacc.Bacc(
        "TRN2", target_bir_lowering=False, debug=False, num_devices=N_CORES
    )

    xq_d = nc.dram_tensor("xq", [D, S_OWN], BF16, kind="ExternalInput")
    wq_d = nc.dram_tensor("wq", [D, D], BF16, kind="ExternalInput")
    wk_d = nc.dram_tensor("wk", [D, D], BF16, kind="ExternalInput")
    wv_d = nc.dram_tensor("wv", [D, D], BF16, kind="ExternalInput")
    wp_d = nc.dram_tensor("wp", [D, D], BF16, kind="ExternalInput")
    fc_d = nc.dram_tensor("fc", [128, FC_W], FP32, kind="ExternalInput")
    bv_d = nc.dram_tensor("bv", [128, D], FP32, kind="ExternalInput")
    bp_d = nc.dram_tensor("bp", [128, D], FP32, kind="ExternalInput")
    ones_d = nc.dram_tensor("ones", [128, 128], FP32R, kind="ExternalInput")
    ones16_d = nc.dram_tensor("ones16", [128, H], BF16, kind="ExternalInput")
    out_d = nc.dram_tensor("out", [S_OWN, D], FP32, kind="ExternalOutput")

    agk_in = nc.dram_tensor("agk_in", [D, S_OWN], BF16)
    agk_out = nc.dram_tensor("agk_out", [GROUP * D, S_OWN], BF16)
    agv_in = nc.dram_tensor("agv_in", [S_OWN, D], BF16)
    agv_out = nc.dram_tensor("agv_out", [S, D], BF16)
    groups = [[0, 1, 2, 3], [4, 5, 6, 7]]

    with tile.TileContext(nc) as tc:
        import contextlib

        with contextlib.ExitStack() as stk:
            # ---- persistent pools --------------------------------------
            w_pool = stk.enter_context(tc.tile_pool(name="w", bufs=1))
            kt_pool = stk.enter_context(tc.tile_pool(name="kt", bufs=1))
            vp_pool = stk.enter_context(tc.tile_pool(name="vp", bufs=1))
            q_pool = stk.enter_context(tc.tile_pool(name="q", bufs=1))
            ao_pool = stk.enter_context(tc.tile_pool(name="ao", bufs=1))
            const_pool = stk.enter_context(tc.tile_pool(name="const", bufs=1))

            # 1024-wide weight tiles: wk(8)+wv(8)+wq(8) live in phase A;
            # wp(8) rotates into wk's slots afterwards
            wk_t = [w_pool.tile([128, D], BF16, name=f"wk{k}", tag="w1024", bufs=24)
                    for k in range(N_KT)]
            wv_t = [w_pool.tile([128, D], BF16, name=f"wv{k}", tag="w1024", bufs=24)
                    for k in range(N_KT)]
            wq_t = [w_pool.tile([128, D], BF16, name=f"wq{k}", tag="w1024", bufs=24)
                    for k in range(N_KT)]

            kT = [kt_pool.tile([128, S], BF16, name=f"kT{p}", tag=f"kT{p}")
                  for p in range(PAIRS)]
            vp = [vp_pool.tile([128, H, HD + 1], BF16, name=f"vp{m}", tag=f"vp{m}")
                  for m in range(N_SK)]
            qA = [q_pool.tile([128, S_OWN], BF16, name=f"qA{p}", tag=f"qA{p}")
                  for p in range(PAIRS)]
            qB = [q_pool.tile([128, S_OWN], BF16, name=f"qB{p}", tag=f"qB{p}")
                  for p in range(PAIRS)]
            aoT = [ao_pool.tile([128, S_OWN], BF16, name=f"ao{p}", tag=f"ao{p}")
                   for p in range(PAIRS)]

            ones_t = const_pool.tile([128, 128], FP32R, tag="ones")
            ones16_t = const_pool.tile([128, H], BF16, tag="ones16")
            fc_t = const_pool.tile([128, FC_W], FP32, tag="fc")
            bv_t = const_pool.tile([128, D], FP32, tag="bv")
            bp_t = const_pool.tile([128, D], FP32, tag="bp")
            actw_t = const_pool.tile([128, 1], FP32, tag="actw")

            nc.gpsimd.dma_start(ones_t[:], ones_d.ap())
            nc.gpsimd.dma_start(fc_t[:], fc_d.ap())
            nc.gpsimd.dma_start(ones16_t[:], ones16_d.ap())

            # ---- PE warm-up + ACT exp-table preload during DMA ramp ----
            with tc.tile_pool(name="warm", bufs=1, space="PSUM") as warm_pool:
                wps = warm_pool.tile([128, 128], FP32, tag="warm")
                for w in range(32):
                    nc.tensor.matmul(
                        wps[:], ones_t[:], ones_t[:],
                        start=True, stop=True, skip_group_check=True,
                    )
            with (
                tc.tile_pool(name="xp", bufs=1) as x_pool,
                tc.tile_pool(name="vsb", bufs=1) as vsb_pool,
                tc.tile_pool(name="vtmp", bufs=4) as vtmp_pool,
                tc.tile_pool(name="psA", bufs=1, space="PSUM") as psA,
            ):
                xq_t = [x_pool.tile([128, S_OWN], BF16, name=f"xq{k}", tag=f"xq{k}")
                        for k in range(N_KT)]
                vsb = [vsb_pool.tile([128, D], BF16, name=f"vsb{m}", tag=f"vsb{m}")
                       for m in range(4)]

                # input streams: xq + wv on sync, wk + wq (+bv/bp, wp) on scalar
                for k in range(N_KT):
                    sl = slice(k * 128, (k + 1) * 128)
                    nc.sync.dma_start(xq_t[k][:], xq_d.ap()[sl, :])
                    nc.scalar.dma_start(wk_t[k][:], wk_d.ap()[sl, :])
                for k in range(N_KT):
                    sl = slice(k * 128, (k + 1) * 128)
                    nc.scalar.dma_start(wv_t[k][:], wv_d.ap()[sl, :])
                for k in range(N_KT):
                    sl = slice(k * 128, (k + 1) * 128)
                    nc.scalar.dma_start(wq_t[k][:], wq_d.ap()[sl, :])
                nc.scalar.dma_start(bv_t[:], bv_d.ap())
                nc.scalar.dma_start(bp_t[:], bp_d.ap())
                nc.scalar.activation(
                    actw_t[:], fc_t[:, FC_MA : FC_MA + 1],
                    mybir.ActivationFunctionType.Exp, scale=0.125,
                )

                ps = [psA.tile([128, 512], FP32, name=f"psA{g}", tag=f"psA{g}")
                      for g in range(8)]

                # -- K wave: kT_part[j] = wk.T @ xq; evac into kT[j][:, 0:512]
                for k in range(N_KT):
                    for j in range(8):
                        nc.tensor.matmul(
                            ps[j][:],
                            wk_t[k][:, j * 128 : (j + 1) * 128],
                            xq_t[k][:],
                            start=(k == 0), stop=(k == N_KT - 1),
                        )
                for j in range(8):
                    nc.vector.tensor_scalar(
                        kT[j][:, 0:S_OWN], ps[j][:],
                        fc_t[:, FC_BK + j : FC_BK + j + 1], None,
                        mybir.AluOpType.add,
                    )
                for j in range(8):
                    nc.sync.dma_start(
                        agk_in.ap()[j * 128 : (j + 1) * 128, :], kT[j][:, 0:S_OWN]
                    )
                nc.gpsimd.collective_compute(
                    "AllGather", mybir.AluOpType.bypass, replica_groups=groups,
                    ins=[agk_in.ap()], outs=[agk_out.ap()],
                )

                # -- V wave: V_own[m] = xq.T @ wv -------------------------
                ps_v = [psA.tile([128, 512], FP32, name=f"psV{g}", tag=f"psA{g}")
                        for g in range(8)]
                for k in range(N_KT):
                    for m in range(4):
                        for nb in range(2):
                            nc.tensor.matmul(
                                ps_v[m * 2 + nb][:],
                                xq_t[k][:, m * 128 : (m + 1) * 128],
                                wv_t[k][:, nb * 512 : (nb + 1) * 512],
                                start=(k == 0), stop=(k == N_KT - 1),
                            )
                for m in range(4):
                    for nb in range(2):
                        nc.vector.tensor_tensor(
                            vsb[m][:, nb * 512 : (nb + 1) * 512],
                            ps_v[m * 2 + nb][:],
                            bv_t[:, nb * 512 : (nb + 1) * 512],
                            mybir.AluOpType.add,
                        )
                for m in range(4):
                    nc.sync.dma_start(
                        agv_in.ap()[m * 128 : (m + 1) * 128, :], vsb[m][:]
                    )
                nc.gpsimd.collective_compute(
                    "AllGather", mybir.AluOpType.bypass, replica_groups=groups,
                    ins=[agv_in.ap()], outs=[agv_out.ap()],
                )

                # own V (local key tiles 0..3): interleave via DVE, no DMA
                for m in range(4):
                    nc.vector.tensor_copy(vp[m][:, :, HD : HD + 1], ones16_t[:])
                    nc.vector.tensor_copy(vp[m][:, :, 0:HD], vsb[m][:])

                # -- Q wave: qT_own[j] = wq.T @ xq, masked/padded halves --
                ps_q = [psA.tile([128, 512], FP32, name=f"psQ{g}", tag=f"psA{g}")
                        for g in range(8)]
                for k in range(N_KT):
                    for j in range(8):
                        nc.tensor.matmul(
                            ps_q[j][:],
                            wq_t[k][:, j * 128 : (j + 1) * 128],
                            xq_t[k][:],
                            start=(k == 0), stop=(k == N_KT - 1),
                        )
                for j in range(8):
                    nc.vector.tensor_scalar(
                        qA[j][:], ps_q[j][:],
                        fc_t[:, FC_MA : FC_MA + 1],
                        fc_t[:, FC_BQA + j : FC_BQA + j + 1],
                        mybir.AluOpType.mult, mybir.AluOpType.add,
                    )
                    nc.vector.tensor_scalar(
                        qB[j][:], ps_q[j][:],
                        fc_t[:, FC_MB : FC_MB + 1],
                        fc_t[:, FC_BQB + j : FC_BQB + j + 1],
                        mybir.AluOpType.mult, mybir.AluOpType.add,
                    )

                # -- gathered loads (runtime-rotated so tiles 0..3 = own) --
                pid = nc.gpsimd.partition_id()
                rank = pid % GROUP
                # kT[p] key block b (b>=1) from group-rank (r+b)%4
                for p in range(PAIRS):
                    for b in range(1, GROUP):
                        row0 = ((rank + b) % GROUP) * D + p * 128
                        nc.gpsimd.dma_start(
                            kT[p][:, b * S_OWN : (b + 1) * S_OWN],
                            agk_out.ap()[bass.ds(row0, 128), :],
                        )
                # V blocks b>=1: contiguous DMA then DVE interleave
                for b in range(1, GROUP):
                    for i in range(4):
                        m = b * 4 + i
                        vt = vtmp_pool.tile([128, D], BF16, name=f"vt{m}", tag="vt")
                        row0 = ((rank + b) % GROUP) * S_OWN + i * 128
                        nc.gpsimd.dma_start(
                            vt[:], agv_out.ap()[bass.ds(row0, 128), :]
                        )
                        nc.vector.tensor_copy(vp[m][:, :, HD : HD + 1], ones16_t[:])
                        nc.vector.tensor_copy(vp[m][:, :, 0:HD], vt[:])

            # wp prefetch (rotates into w1024 slots)
            wp_t = [w_pool.tile([128, D], BF16, name=f"wp{k}", tag="w1024", bufs=24)
                    for k in range(N_KT)]
            for k in range(N_KT):
                nc.scalar.dma_start(wp_t[k][:], wp_d.ap()[k * 128 : (k + 1) * 128, :])

            # ---- phase B: attention ------------------------------------
            # sweep 1: own key tiles (0..3) for every pair -- runs while the
            # gathers are in flight; PSUM accs partial-evac'd to SBUF.
            # sweep 2: gathered tiles (4..15), partial re-added, normalize.
            with (
                tc.tile_pool(name="p", bufs=14) as p_pool,
                tc.tile_pool(name="part", bufs=1) as part_pool,
                tc.tile_pool(name="rr", bufs=2) as rr_pool,
                tc.tile_pool(name="rcp", bufs=2) as rcp_pool,
                tc.tile_pool(name="psc", bufs=2, space="PSUM") as ps_sc,
                tc.tile_pool(name="pacc", bufs=2, space="PSUM") as ps_acc,
            ):
                part_a = [part_pool.tile([128, S_OWN], BF16, name=f"pa{p}", tag=f"pa{p}")
                          for p in range(PAIRS)]
                part_b = [part_pool.tile([128, S_OWN], BF16, name=f"pb{p}", tag=f"pb{p}")
                          for p in range(PAIRS)]

                def attn_block(p, tp, acc_a, acc_b, first_tp, last_tp):
                    t0, t1 = 2 * tp, 2 * tp + 1
                    sca = ps_sc.tile([128, 1024], FP32, tag="sc", name=f"sca{p}_{tp}")
                    scb = ps_sc.tile([128, 1024], FP32, tag="sc", name=f"scb{p}_{tp}")
                    for ti, t in enumerate((t0, t1)):
                        tsl = slice(t * 128, (t + 1) * 128)
                        usl = slice(ti * 512, (ti + 1) * 512)
                        nc.tensor.matmul(
                            sca[:, usl], kT[p][:, tsl], qA[p][:],
                            start=True, stop=True,
                        )
                        nc.tensor.matmul(
                            scb[:, usl], kT[p][:, tsl], qB[p][:],
                            start=True, stop=True,
                        )
                    pa = p_pool.tile([128, 1024], BF16, tag="pt", name=f"pta{p}_{tp}")
                    pb = p_pool.tile([128, 1024], BF16, tag="pt", name=f"ptb{p}_{tp}")
                    nc.scalar.activation(
                        pa[:], sca[:], mybir.ActivationFunctionType.Exp, scale=0.125
                    )
                    nc.scalar.activation(
                        pb[:], scb[:], mybir.ActivationFunctionType.Exp, scale=0.125
                    )
                    for ti, t in enumerate((t0, t1)):
                        usl = slice(ti * 512, (ti + 1) * 512)
                        first = tp == first_tp and ti == 0
                        last = tp == last_tp and ti == 1
                        nc.tensor.matmul(
                            acc_a[0:65, :], vp[t][:, 2 * p : 2 * p + 1, :],
                            pa[:, usl], start=first, stop=last,
                        )
                        nc.tensor.matmul(
                            acc_b[0:65, :], vp[t][:, 2 * p + 1 : 2 * p + 2, :],
                            pb[:, usl], start=first, stop=last,
                        )

                # sweep 1: local key tiles
                for p in range(PAIRS):
                    acc_a = ps_acc.tile([128, S_OWN], FP32, tag="acca", name=f"a1a{p}")
                    acc_b = ps_acc.tile([128, S_OWN], FP32, tag="accb", name=f"a1b{p}")
                    for tp in range(2):
                        attn_block(p, tp, acc_a, acc_b, 0, 1)
                    nc.vector.tensor_copy(part_a[p][0:65, :], acc_a[0:65, :])
                    nc.vector.tensor_copy(part_b[p][0:65, :], acc_b[0:65, :])

                # sweep 2: gathered key tiles + combine + normalize
                for p in range(PAIRS):
                    acc_a = ps_acc.tile([128, S_OWN], FP32, tag="acca", name=f"a2a{p}")
                    acc_b = ps_acc.tile([128, S_OWN], FP32, tag="accb", name=f"a2b{p}")
                    for tp in range(2, N_SK // 2):
                        attn_block(p, tp, acc_a, acc_b, 2, N_SK // 2 - 1)
                    nc.vector.tensor_tensor(
                        acc_a[0:65, :], acc_a[0:65, :], part_a[p][0:65, :],
                        mybir.AluOpType.add,
                    )
                    nc.vector.tensor_tensor(
                        acc_b[0:65, :], acc_b[0:65, :], part_b[p][0:65, :],
                        mybir.AluOpType.add,
                    )
                    # normalize both halves with one reciprocal
                    rrow = rr_pool.tile([1, 1024], FP32R, tag="rrow")
                    nc.vector.tensor_copy(rrow[:, 0:512], acc_a[64:65, :])
                    nc.vector.tensor_copy(rrow[:, 512:1024], acc_b[64:65, :])
                    rb = ps_sc.tile([64, 1024], FP32, tag="sc", name=f"rb{p}")
                    nc.tensor.matmul(
                        rb[:, 0:512], ones_t[0:1, 0:64], rrow[:, 0:512],
                        start=True, stop=True,
                    )
                    nc.tensor.matmul(
                        rb[:, 512:1024], ones_t[0:1, 0:64], rrow[:, 512:1024],
                        start=True, stop=True,
                    )
                    rc = rcp_pool.tile([64, 1024], FP32, tag="rc")
                    nc.vector.reciprocal_approx_fast(rc[:], rb[:])
                    nc.vector.tensor_tensor(
                        aoT[p][0:64, :], acc_a[0:64, :], rc[:, 0:512],
                        mybir.AluOpType.mult,
                    )
                    nc.vector.tensor_tensor(
                        aoT[p][64:128, :], acc_b[0:64, :], rc[:, 512:1024],
                        mybir.AluOpType.mult,
                    )

            # ---- phase C: local output projection ----------------------
            with (
                tc.tile_pool(name="outp", bufs=4) as out_pool,
                tc.tile_pool(name="psD", bufs=1, space="PSUM") as psD,
            ):
                for m in range(4):
                    msl = slice(m * 128, (m + 1) * 128)
                    for nb in range(2):
                        nsl = slice(nb * 512, (nb + 1) * 512)
                        pd = psD.tile([128, 512], FP32, tag=f"psD{m * 2 + nb}")
                        for kd in range(N_KT):
                            nc.tensor.matmul(
                                pd[:],
                                aoT[kd][:, msl],
                                wp_t[kd][:, nsl],
                                start=(kd == 0), stop=(kd == N_KT - 1),
                            )
                        ot = out_pool.tile([128, 512], FP32, tag="ot")
                        nc.vector.tensor_tensor(
                            ot[:], pd[:], bp_t[:, nsl], mybir.AluOpType.add
                        )
                        nc.sync.dma_start(out_d.ap()[msl, nsl], ot[:])

    nc.compile()
    return nc


def _get_program():
    global _compiled
    if _compiled is None:
        _compiled = _build()
    return _compiled


def _make_in_maps(x, w_qkv, b_qkv, w_proj, b_proj):
    x = np.asarray(x, dtype=np.float32)
    w_qkv = np.asarray(w_qkv, dtype=np.float32)
    b_qkv = np.asarray(b_qkv, dtype=np.float32)
    w_proj = np.asarray(w_proj, dtype=np.float32)
    b_proj = np.asarray(b_proj, dtype=np.float32)

    global _ONES16
    if _ONES16 is None:
        _ONES16 = np.ones((128, H), dtype=BF16_NP)

    wq16 = np.ascontiguousarray(w_qkv[:, 0:D]).astype(BF16_NP)
    wk16 = np.ascontiguousarray(w_qkv[:, D : 2 * D]).astype(BF16_NP)
    wv16 = np.ascontiguousarray(w_qkv[:, 2 * D : 3 * D]).astype(BF16_NP)
    wp16 = w_proj.astype(BF16_NP)

    fc = np.zeros((128, FC_W), dtype=np.float32)
    bq = b_qkv[0:D]
    bk = b_qkv[D : 2 * D]
    for j in range(PAIRS):
        fc[0:64, FC_BQA + j] = bq[j * 128 : j * 128 + 64]
        fc[64:128, FC_BQB + j] = bq[j * 128 + 64 : (j + 1) * 128]
        fc[:, FC_BK + j] = bk[j * 128 : (j + 1) * 128]
    fc[0:64, FC_MA] = 1.0
    fc[64:128, FC_MB] = 1.0
    bv_b = np.ascontiguousarray(
        np.broadcast_to(b_qkv[2 * D : 3 * D].reshape(1, D), (128, D))
    )
    bp_b = np.ascontiguousarray(np.broadcast_to(b_proj.reshape(1, D), (128, D)))

    xT = [np.ascontiguousarray(x[g].T).astype(BF16_NP) for g in range(B)]
    in_maps = []
    for c in range(N_CORES):
        g, r = c // GROUP, c % GROUP
        in_maps.append(
            {
                "xq": np.ascontiguousarray(
                    xT[g][:, r * S_OWN : (r + 1) * S_OWN]
                ),
                "wq": wq16,
                "wk": wk16,
                "wv": wv16,
                "wp": wp16,
                "fc": fc,
                "bv": bv_b,
                "bp": bp_b,
                "ones": _ONES,
                "ones16": _ONES16,
            }
        )
    return in_maps


def _assemble(results):
    out = np.empty((B, S, D), dtype=np.float32)
    for c in range(N_CORES):
        g, r = c // GROUP, c % GROUP
        out[g, r * S_OWN : (r + 1) * S_OWN, :] = results[c]["out"]
    return out


def kernel(x, w_qkv, b_qkv, w_proj, b_proj):
    nc = _get_program()
    in_maps = _make_in_maps(x, w_qkv, b_qkv, w_proj, b_proj)
    res = run_bass_kernel_spmd(nc, in_maps, list(range(N_CORES)))
    return _assemble(res.results)
